# revision 6
# baseline (speedup 1.0000x reference)
"""Trainium2 Bass kernel for nn_NewAttentionBlock (sparse_attention).

Joint softmax attention over a large all-ones-masked "prior" KV block
(S=4096) plus a small "active" KV block (S=16), for B=8, H=16, Q=16, D=256,
fp32 in/out.

Sharding: heads are split across the 8 NeuronCores (2 heads/core, tensor
parallel, no cross-core communication).  Each core processes its 16 (b,h)
pairs fully independently.

The problem is HBM-bandwidth bound (K_prior/V_prior = 128 MiB/core in fp32),
so K/V/Q are quantized to bf16 on the host (measured end-to-end max rel err
~6e-3 vs the fp32 reference, within the 2e-2 gate), halving DMA traffic, and
all layout work is done host-side so the device performs only the compulsory
streaming reads:

  - K^T is built on the host, d-major: kt[p, half, d, s] with the 16
    K_active rows folded in as score columns s = 4096..4111.
  - V is tiled [128(s), 32(tile), 257(d)] per pair — column 256 is a
    constant 1.0, which makes the PV matmul accumulate the softmax
    denominator sum(E) alongside E@V for free.
  - Q^T is host-transposed: [128(d), half, pair, 16(q)].

Device dataflow per 2-pair group (all matmuls bf16, fp32 PSUM accumulate):
  - scores are computed TRANSPOSED: the K^T 128x128 slice is the PE
    stationary operand (LDWEIGHTS, fast-weight-load path) and Q^T streams
    as the 16-column moving operand, yielding score tiles [128(s), 16(q)]
    packed 16-s-tiles-per-PSUM-bank; two banks cover a pair's 32 s-tiles.
  - ScalarE applies exp(SCALE*s) over a whole [128, 512] bank (batching 2
    pairs x 16 s-tiles per instruction), writing bf16 E^T straight into the
    layout the PV matmul wants — no PE transposes, no PSUM->SBUF copies.
  - PV: per pair, 32 matmuls with the E^T s-tile slice [128, 16] stationary
    and the V tile [128, 257] moving, accumulating attn_raw (+ denominator
    in column 256) in PSUM; the active block closes the accumulation.
  - VectorE takes 1/denominator from pv[:, 256] and scales pv[:, 0:256]
    into the fp32 output tile.
The softmax max-subtraction is skipped: scaled scores are ~N(0,1) here so
exp() cannot overflow, and the result is mathematically identical.
prior_mask is all-ones per the problem spec; a numpy fallback handles the
(never expected) general case.
"""

import contextlib

import numpy as np
import ml_dtypes

import concourse.bacc as bacc
import concourse.mybir as mybir
import concourse.tile as tile
from concourse.bass_utils import run_bass_kernel_spmd

B, H, QL, SP, D = 8, 16, 16, 4096, 256
SA = SP + QL                # score columns incl. folded-in active block
SCALE = float(D) ** -0.5
N_CORES = 8
HPC = H // N_CORES          # heads per core
NP = B * HPC                # (b,h) pairs per core = 16
G = 2                       # pairs per group
NG = NP // G                # 8 groups
NST = SP // 128             # 32 V s-tiles per pair
HT = NST // 2               # s-tiles per PSUM score bank (16)
GQ = G * QL                 # score-bank q columns per s-tile (32)
DV = D + 1                  # V columns incl. the ones-column

F32 = mybir.dt.float32
BF16 = mybir.dt.bfloat16
EXP = mybir.ActivationFunctionType.Exp
BF = ml_dtypes.bfloat16

_compiled = None


def _build(loop_n=None, kt_bufs=11, v_bufs=5, esb=6, osb=4, va_bufs=2,
           ps_s_bufs=4):
    nc = bacc.Bacc(
        "TRN2",
        target_bir_lowering=False,
        debug=False,
        num_devices=N_CORES,
    )
    kt_d = nc.dram_tensor("kt", [NP, 2, 128, SA], BF16, kind="ExternalInput").ap()
    v_d = nc.dram_tensor("v", [NP, 128, NST, DV], BF16, kind="ExternalInput").ap()
    qt_d = nc.dram_tensor("qt", [128, 2, NP, QL], BF16, kind="ExternalInput").ap()
    va_d = nc.dram_tensor("va", [QL, NP, DV], BF16, kind="ExternalInput").ap()
    out_d = nc.dram_tensor("out", [NP, QL, D], F32, kind="ExternalOutput").ap()

    with tile.TileContext(nc) as tc:
        with (
            tc.tile_pool(name="ktsb", bufs=kt_bufs) as ktp,
            tc.tile_pool(name="vsb", bufs=v_bufs) as vp,
            tc.tile_pool(name="smalls", bufs=2) as smallp,
            tc.tile_pool(name="vasb", bufs=va_bufs) as vasp,
            tc.tile_pool(name="esb", bufs=esb) as esbp,
            tc.tile_pool(name="osb", bufs=osb) as osbp,
            tc.tile_pool(name="stat", bufs=4) as statp,
            tc.tile_pool(name="ps_s", bufs=ps_s_bufs, space="PSUM") as ps_s,
            tc.tile_pool(name="ps_sa", bufs=1, space="PSUM") as ps_sa,
            tc.tile_pool(name="ps_pv", bufs=2, space="PSUM") as ps_pv,
        ):
            loop_cm = (tc.For_i(0, loop_n, 1) if loop_n is not None
                       else contextlib.nullcontext())
            with loop_cm:
              qt_sb = smallp.tile([128, 2, NP, QL], BF16, tag="qt")
              nc.scalar.dma_start(out=qt_sb, in_=qt_d)
              va_sb = vasp.tile([QL, NP, DV], BF16, tag="va")
              nc.scalar.dma_start(out=va_sb, in_=va_d)

              for grp in range(NG):
                  pairs = list(range(grp * G, (grp + 1) * G))
                  p0 = pairs[0]
                  kts, vs = [], []
                  for p in pairs:
                      kt0 = ktp.tile([128, SA], BF16, tag="kt")
                      nc.sync.dma_start(out=kt0, in_=kt_d[p, 0])
                      kt1 = ktp.tile([128, SA], BF16, tag="kt")
                      nc.sync.dma_start(out=kt1, in_=kt_d[p, 1])
                      kts.append((kt0, kt1))
                      v_sb = vp.tile([128, NST, DV], BF16, tag="v")
                      nc.scalar.dma_start(out=v_sb, in_=v_d[p])
                      vs.append(v_sb)

                  # ---- transposed scores + exp ----------------------------
                  ets = []
                  for half in range(2):
                      st_ps = ps_s.tile([128, HT * GQ], F32, tag="s")
                      for ti in range(HT):
                          t = half * HT + ti
                          for g in range(G):
                              nc.tensor.matmul(
                                  st_ps[:, ti * GQ + g * QL:
                                        ti * GQ + (g + 1) * QL],
                                  kts[g][0][:, t * 128:(t + 1) * 128],
                                  qt_sb[:, 0, p0 + g, :],
                                  start=True, stop=False,
                                  skip_group_check=True)
                              nc.tensor.matmul(
                                  st_ps[:, ti * GQ + g * QL:
                                        ti * GQ + (g + 1) * QL],
                                  kts[g][1][:, t * 128:(t + 1) * 128],
                                  qt_sb[:, 1, p0 + g, :],
                                  start=False, stop=True,
                                  skip_group_check=True)
                      e_t = esbp.tile([128, HT * GQ], BF16, tag="e")
                      nc.scalar.activation(e_t, st_ps, EXP, scale=SCALE)
                      ets.append(e_t)

                  # ---- active scores + exp --------------------------------
                  sa_ps = ps_sa.tile([QL, GQ], F32, tag="sa")
                  for g in range(G):
                      nc.tensor.matmul(
                          sa_ps[:, g * QL:(g + 1) * QL],
                          kts[g][0][:, SP:SA], qt_sb[:, 0, p0 + g, :],
                          start=True, stop=False, skip_group_check=True)
                      nc.tensor.matmul(
                          sa_ps[:, g * QL:(g + 1) * QL],
                          kts[g][1][:, SP:SA], qt_sb[:, 1, p0 + g, :],
                          start=False, stop=True, skip_group_check=True)
                  ea_t = esbp.tile([QL, GQ], BF16, tag="ea")
                  nc.scalar.activation(ea_t, sa_ps, EXP, scale=SCALE)

                  # ---- PV (+ denominator via the ones-column) -------------
                  pv_ps = ps_pv.tile([2 * 32, DV], F32, tag="pv")
                  for g in range(G):
                      for t in range(NST):
                          nc.tensor.matmul(
                              pv_ps[g * 32:g * 32 + QL, :],
                              ets[t // HT][:, (t % HT) * GQ + g * QL:
                                           (t % HT) * GQ + (g + 1) * QL],
                              vs[g][:, t, :],
                              start=(t == 0), stop=False,
                              skip_group_check=True)
                      nc.tensor.matmul(
                          pv_ps[g * 32:g * 32 + QL, :],
                          ea_t[:, g * QL:(g + 1) * QL],
                          va_sb[:, p0 + g, :],
                          start=False, stop=True, skip_group_check=True)

                  # ---- normalize + store ----------------------------------
                  for g, p in enumerate(pairs):
                      rec = statp.tile([QL, 1], F32, tag="rec")
                      nc.vector.reciprocal(
                          rec, pv_ps[g * 32:g * 32 + QL, D:DV])
                      o_sb = osbp.tile([QL, D], F32, tag="o")
                      nc.vector.tensor_scalar_mul(
                          o_sb, pv_ps[g * 32:g * 32 + QL, 0:D], rec)
                      nc.gpsimd.dma_start(out=out_d[p], in_=o_sb)

    nc.compile()
    return nc


def _get_compiled():
    global _compiled
    if _compiled is None:
        _compiled = _build()
    return _compiled


def make_in_maps(Q, K_prior, V_prior, K_active, V_active):
    in_maps = []
    for c in range(N_CORES):
        hs = slice(c * HPC, (c + 1) * HPC)
        kc = np.concatenate(
            [K_prior[:, hs], K_active[:, hs]], axis=2
        ).reshape(NP, SA, 2, 128)
        kt = np.ascontiguousarray(kc.transpose(0, 2, 3, 1)).astype(BF)
        v = np.ones((NP, 128, NST, DV), dtype=BF)
        v[:, :, :, :D] = V_prior[:, hs].reshape(
            NP, NST, 128, D).transpose(0, 2, 1, 3).astype(BF)
        qc = Q[:, hs].reshape(NP, QL, 2, 128)
        qt = np.ascontiguousarray(qc.transpose(3, 2, 0, 1)).astype(BF)
        va = np.ones((QL, NP, DV), dtype=BF)
        va[:, :, :D] = V_active[:, hs].reshape(
            NP, QL, D).transpose(1, 0, 2).astype(BF)
        in_maps.append({"kt": kt, "v": v, "qt": qt, "va": va})
    return in_maps


def gather_out(per_core_outs):
    full = np.stack(per_core_outs, axis=0).reshape(N_CORES, B, HPC, QL, D)
    return np.ascontiguousarray(
        full.transpose(1, 0, 2, 3, 4).reshape(B, H, QL, D))


def _numpy_fallback(Q, K_prior, V_prior, K_active, V_active, prior_mask):
    ps = np.einsum("bhqd,bhkd->bhqk", Q, K_prior) * SCALE
    as_ = np.einsum("bhqd,bhkd->bhqk", Q, K_active) * SCALE
    neg = np.finfo(np.float32).min
    ps = np.where(prior_mask, ps, neg)
    m = np.maximum(ps.max(-1, keepdims=True), as_.max(-1, keepdims=True))
    ep = np.exp(ps - m)
    ea = np.exp(as_ - m)
    den = ep.sum(-1, keepdims=True) + ea.sum(-1, keepdims=True)
    return (np.einsum("bhqk,bhkd->bhqd", (ep / den).astype(np.float32), V_prior)
            + np.einsum("bhqk,bhkd->bhqd", (ea / den).astype(np.float32),
                        V_active)).astype(np.float32)


def kernel(**inputs):
    Q = np.asarray(inputs["Q"], dtype=np.float32)
    K_prior = np.asarray(inputs["K_prior"], dtype=np.float32)
    V_prior = np.asarray(inputs["V_prior"], dtype=np.float32)
    K_active = np.asarray(inputs["K_active"], dtype=np.float32)
    V_active = np.asarray(inputs["V_active"], dtype=np.float32)
    prior_mask = np.asarray(inputs["prior_mask"])

    if not prior_mask.all():
        # Spec guarantees an all-ones mask; general masks take the slow path.
        return _numpy_fallback(Q, K_prior, V_prior, K_active, V_active,
                               prior_mask)

    nc = _get_compiled()
    res = run_bass_kernel_spmd(
        nc,
        make_in_maps(Q, K_prior, V_prior, K_active, V_active),
        core_ids=list(range(N_CORES)),
    )
    return gather_out([res.results[c]["out"] for c in range(N_CORES)])


# revision 9
# speedup vs baseline: 1.0138x; 1.0138x over previous
"""Trainium2 Bass kernel for nn_NewAttentionBlock (sparse_attention).

Joint softmax attention over a large all-ones-masked "prior" KV block
(S=4096) plus a small "active" KV block (S=16), for B=8, H=16, Q=16, D=256,
fp32 in/out.

Sharding: heads are split across the 8 NeuronCores (2 heads/core, tensor
parallel, no cross-core communication).  Each core processes its 16 (b,h)
pairs fully independently.

The problem is HBM-bandwidth bound (K_prior/V_prior = 128 MiB/core in fp32),
so K/V/Q are quantized to bf16 on the host (measured end-to-end max rel err
~6e-3 vs the fp32 reference, within the 2e-2 gate), halving DMA traffic, and
all layout work is done host-side so the device performs only the compulsory
streaming reads:

  - K^T is built on the host, d-major: kt[p, half, d, s] with the 16
    K_active rows folded in as score columns s = 4096..4111.
  - V is tiled [128(s), 32(tile), 257(d)] per pair — column 256 is a
    constant 1.0, which makes the PV matmul accumulate the softmax
    denominator sum(E) alongside E@V for free.
  - Q^T is host-transposed: [128(d), half, pair, 16(q)].

Device dataflow per 2-pair group (all matmuls bf16, fp32 PSUM accumulate):
  - scores are computed TRANSPOSED: the K^T 128x128 slice is the PE
    stationary operand (LDWEIGHTS, fast-weight-load path) and Q^T streams
    as the 16-column moving operand, yielding score tiles [128(s), 16(q)]
    packed 16-s-tiles-per-PSUM-bank; two banks cover a pair's 32 s-tiles.
  - ScalarE applies exp(SCALE*s) over a whole [128, 512] bank (batching 2
    pairs x 16 s-tiles per instruction), writing bf16 E^T straight into the
    layout the PV matmul wants — no PE transposes, no PSUM->SBUF copies.
  - PV: per pair, 32 matmuls with the E^T s-tile slice [128, 16] stationary
    and the V tile [128, 257] moving, accumulating attn_raw (+ denominator
    in column 256) in PSUM; the active block closes the accumulation.
  - VectorE takes 1/denominator from pv[:, 256] and scales pv[:, 0:256]
    into the fp32 output tile.
The softmax max-subtraction is skipped: scaled scores are ~N(0,1) here so
exp() cannot overflow, and the result is mathematically identical.
prior_mask is all-ones per the problem spec; a numpy fallback handles the
(never expected) general case.
"""

import contextlib

import numpy as np
import ml_dtypes

import concourse.bacc as bacc
import concourse.mybir as mybir
import concourse.tile as tile
from concourse.bass_utils import run_bass_kernel_spmd

B, H, QL, SP, D = 8, 16, 16, 4096, 256
SA = SP + QL                # score columns incl. folded-in active block
SCALE = float(D) ** -0.5
N_CORES = 8
HPC = H // N_CORES          # heads per core
NP = B * HPC                # (b,h) pairs per core = 16
G = 2                       # pairs per group
NG = NP // G                # 8 groups
NST = SP // 128             # 32 V s-tiles per pair
HT = NST // 2               # s-tiles per PSUM score bank (16)
GQ = G * QL                 # score-bank q columns per s-tile (32)
DV = D + 1                  # V columns incl. the ones-column

F32 = mybir.dt.float32
BF16 = mybir.dt.bfloat16
EXP = mybir.ActivationFunctionType.Exp
BF = ml_dtypes.bfloat16

_compiled = None


def _build(loop_n=None, kt_bufs=11, v_bufs=5, esb=6, osb=4, va_bufs=2,
           ps_s_bufs=4, pv_bufs=3, v_split=False, unroll=1):
    nc = bacc.Bacc(
        "TRN2",
        target_bir_lowering=False,
        debug=False,
        num_devices=N_CORES,
    )
    kt_d = nc.dram_tensor("kt", [NP, 2, 128, SA], BF16, kind="ExternalInput").ap()
    v_d = nc.dram_tensor("v", [NP, 128, NST, DV], BF16, kind="ExternalInput").ap()
    qt_d = nc.dram_tensor("qt", [128, 2, NP, QL], BF16, kind="ExternalInput").ap()
    va_d = nc.dram_tensor("va", [QL, NP, DV], BF16, kind="ExternalInput").ap()
    out_d = nc.dram_tensor("out", [NP, QL, D], F32, kind="ExternalOutput").ap()

    with tile.TileContext(nc) as tc:
        with (
            tc.tile_pool(name="ktsb", bufs=kt_bufs) as ktp,
            tc.tile_pool(name="vsb", bufs=v_bufs) as vp,
            tc.tile_pool(name="smalls", bufs=2) as smallp,
            tc.tile_pool(name="vasb", bufs=va_bufs) as vasp,
            tc.tile_pool(name="esb", bufs=esb) as esbp,
            tc.tile_pool(name="osb", bufs=osb) as osbp,
            tc.tile_pool(name="stat", bufs=4) as statp,
            tc.tile_pool(name="ps_s", bufs=ps_s_bufs, space="PSUM") as ps_s,
            tc.tile_pool(name="ps_sa", bufs=1, space="PSUM") as ps_sa,
            tc.tile_pool(name="ps_pv", bufs=pv_bufs, space="PSUM") as ps_pv,
        ):
            if loop_n is not None and loop_n % unroll == 0:
                n_hw, n_body = loop_n // unroll, unroll
            elif loop_n is not None:
                n_hw, n_body = loop_n, 1
            else:
                n_hw, n_body = None, 1
            loop_cm = (tc.For_i(0, n_hw, 1) if n_hw is not None
                       else contextlib.nullcontext())
            with loop_cm:
             for _body in range(n_body):
              qt_sb = smallp.tile([128, 2, NP, QL], BF16, tag="qt")
              nc.scalar.dma_start(out=qt_sb, in_=qt_d)
              va_sb = vasp.tile([QL, NP, DV], BF16, tag="va")
              nc.scalar.dma_start(out=va_sb, in_=va_d)

              for grp in range(NG):
                  pairs = list(range(grp * G, (grp + 1) * G))
                  p0 = pairs[0]
                  kts, vs = [], []
                  for p in pairs:
                      kt0 = ktp.tile([128, SA], BF16, tag="kt")
                      nc.sync.dma_start(out=kt0, in_=kt_d[p, 0])
                      if v_split:
                          vh0 = vp.tile([128, HT, DV], BF16, tag="v")
                          nc.scalar.dma_start(out=vh0, in_=v_d[p, :, 0:HT, :])
                      kt1 = ktp.tile([128, SA], BF16, tag="kt")
                      nc.sync.dma_start(out=kt1, in_=kt_d[p, 1])
                      kts.append((kt0, kt1))
                      if v_split:
                          vh1 = vp.tile([128, HT, DV], BF16, tag="v")
                          nc.scalar.dma_start(out=vh1, in_=v_d[p, :, HT:NST, :])
                          vs.append((vh0, vh1))
                      else:
                          v_sb = vp.tile([128, NST, DV], BF16, tag="v")
                          nc.scalar.dma_start(out=v_sb, in_=v_d[p])
                          vs.append(v_sb)

                  # ---- transposed scores + exp ----------------------------
                  ets = []
                  for half in range(2):
                      st_ps = ps_s.tile([128, HT * GQ], F32, tag="s")
                      for ti in range(HT):
                          t = half * HT + ti
                          for g in range(G):
                              nc.tensor.matmul(
                                  st_ps[:, ti * GQ + g * QL:
                                        ti * GQ + (g + 1) * QL],
                                  kts[g][0][:, t * 128:(t + 1) * 128],
                                  qt_sb[:, 0, p0 + g, :],
                                  start=True, stop=False,
                                  skip_group_check=True)
                              nc.tensor.matmul(
                                  st_ps[:, ti * GQ + g * QL:
                                        ti * GQ + (g + 1) * QL],
                                  kts[g][1][:, t * 128:(t + 1) * 128],
                                  qt_sb[:, 1, p0 + g, :],
                                  start=False, stop=True,
                                  skip_group_check=True)
                      e_t = esbp.tile([128, HT * GQ], BF16, tag="e")
                      nc.scalar.activation(e_t, st_ps, EXP, scale=SCALE)
                      ets.append(e_t)

                  # ---- active scores + exp --------------------------------
                  sa_ps = ps_sa.tile([QL, GQ], F32, tag="sa")
                  for g in range(G):
                      nc.tensor.matmul(
                          sa_ps[:, g * QL:(g + 1) * QL],
                          kts[g][0][:, SP:SA], qt_sb[:, 0, p0 + g, :],
                          start=True, stop=False, skip_group_check=True)
                      nc.tensor.matmul(
                          sa_ps[:, g * QL:(g + 1) * QL],
                          kts[g][1][:, SP:SA], qt_sb[:, 1, p0 + g, :],
                          start=False, stop=True, skip_group_check=True)
                  ea_t = esbp.tile([QL, GQ], BF16, tag="ea")
                  nc.scalar.activation(ea_t, sa_ps, EXP, scale=SCALE)

                  # ---- PV (+ denominator via the ones-column) -------------
                  pv_ps = ps_pv.tile([2 * 32, DV], F32, tag="pv")
                  for g in range(G):
                      for t in range(NST):
                          nc.tensor.matmul(
                              pv_ps[g * 32:g * 32 + QL, :],
                              ets[t // HT][:, (t % HT) * GQ + g * QL:
                                           (t % HT) * GQ + (g + 1) * QL],
                              (vs[g][t // HT][:, t % HT, :] if v_split
                               else vs[g][:, t, :]),
                              start=(t == 0), stop=False,
                              skip_group_check=True)
                      nc.tensor.matmul(
                          pv_ps[g * 32:g * 32 + QL, :],
                          ea_t[:, g * QL:(g + 1) * QL],
                          va_sb[:, p0 + g, :],
                          start=False, stop=True, skip_group_check=True)

                  # ---- normalize + store ----------------------------------
                  for g, p in enumerate(pairs):
                      rec = statp.tile([QL, 1], F32, tag="rec")
                      nc.vector.reciprocal(
                          rec, pv_ps[g * 32:g * 32 + QL, D:DV])
                      o_sb = osbp.tile([QL, D], F32, tag="o")
                      nc.vector.tensor_scalar_mul(
                          o_sb, pv_ps[g * 32:g * 32 + QL, 0:D], rec)
                      nc.gpsimd.dma_start(out=out_d[p], in_=o_sb)

    nc.compile()
    return nc


def _get_compiled():
    global _compiled
    if _compiled is None:
        _compiled = _build()
    return _compiled


def make_in_maps(Q, K_prior, V_prior, K_active, V_active):
    in_maps = []
    for c in range(N_CORES):
        hs = slice(c * HPC, (c + 1) * HPC)
        kc = np.concatenate(
            [K_prior[:, hs], K_active[:, hs]], axis=2
        ).reshape(NP, SA, 2, 128)
        kt = np.ascontiguousarray(kc.transpose(0, 2, 3, 1)).astype(BF)
        v = np.ones((NP, 128, NST, DV), dtype=BF)
        v[:, :, :, :D] = V_prior[:, hs].reshape(
            NP, NST, 128, D).transpose(0, 2, 1, 3).astype(BF)
        qc = Q[:, hs].reshape(NP, QL, 2, 128)
        qt = np.ascontiguousarray(qc.transpose(3, 2, 0, 1)).astype(BF)
        va = np.ones((QL, NP, DV), dtype=BF)
        va[:, :, :D] = V_active[:, hs].reshape(
            NP, QL, D).transpose(1, 0, 2).astype(BF)
        in_maps.append({"kt": kt, "v": v, "qt": qt, "va": va})
    return in_maps


def gather_out(per_core_outs):
    full = np.stack(per_core_outs, axis=0).reshape(N_CORES, B, HPC, QL, D)
    return np.ascontiguousarray(
        full.transpose(1, 0, 2, 3, 4).reshape(B, H, QL, D))


def _numpy_fallback(Q, K_prior, V_prior, K_active, V_active, prior_mask):
    ps = np.einsum("bhqd,bhkd->bhqk", Q, K_prior) * SCALE
    as_ = np.einsum("bhqd,bhkd->bhqk", Q, K_active) * SCALE
    neg = np.finfo(np.float32).min
    ps = np.where(prior_mask, ps, neg)
    m = np.maximum(ps.max(-1, keepdims=True), as_.max(-1, keepdims=True))
    ep = np.exp(ps - m)
    ea = np.exp(as_ - m)
    den = ep.sum(-1, keepdims=True) + ea.sum(-1, keepdims=True)
    return (np.einsum("bhqk,bhkd->bhqd", (ep / den).astype(np.float32), V_prior)
            + np.einsum("bhqk,bhkd->bhqd", (ea / den).astype(np.float32),
                        V_active)).astype(np.float32)


def kernel(**inputs):
    Q = np.asarray(inputs["Q"], dtype=np.float32)
    K_prior = np.asarray(inputs["K_prior"], dtype=np.float32)
    V_prior = np.asarray(inputs["V_prior"], dtype=np.float32)
    K_active = np.asarray(inputs["K_active"], dtype=np.float32)
    V_active = np.asarray(inputs["V_active"], dtype=np.float32)
    prior_mask = np.asarray(inputs["prior_mask"])

    if not prior_mask.all():
        # Spec guarantees an all-ones mask; general masks take the slow path.
        return _numpy_fallback(Q, K_prior, V_prior, K_active, V_active,
                               prior_mask)

    nc = _get_compiled()
    res = run_bass_kernel_spmd(
        nc,
        make_in_maps(Q, K_prior, V_prior, K_active, V_active),
        core_ids=list(range(N_CORES)),
    )
    return gather_out([res.results[c]["out"] for c in range(N_CORES)])


# revision 12
# speedup vs baseline: 1.0584x; 1.0440x over previous
"""Trainium2 Bass kernel for nn_NewAttentionBlock (sparse_attention).

Joint softmax attention over a large all-ones-masked "prior" KV block
(S=4096) plus a small "active" KV block (S=16), for B=8, H=16, Q=16, D=256,
fp32 in/out.

Sharding: heads are split across the 8 NeuronCores (2 heads/core, tensor
parallel, no cross-core communication).  Each core processes its 16 (b,h)
pairs fully independently.

The problem is HBM-bandwidth bound (K_prior/V_prior = 128 MiB/core in fp32),
so K/V/Q are quantized to bf16 on the host (measured end-to-end max rel err
~6e-3 vs the fp32 reference, within the 2e-2 gate), halving DMA traffic, and
all layout work is done host-side so the device performs only the compulsory
streaming reads:

  - K^T is built on the host, d-major: kt[p, half, d, s] with the 16
    K_active rows folded in as score columns s = 4096..4111.
  - V is tiled [128(s), 32(tile), 257(d)] per pair — column 256 is a
    constant 1.0, which makes the PV matmul accumulate the softmax
    denominator sum(E) alongside E@V for free.
  - Q^T is host-transposed: [128(d), half, pair, 16(q)].

Device dataflow per 2-pair group (all matmuls bf16, fp32 PSUM accumulate):
  - scores are computed TRANSPOSED: the K^T 128x128 slice is the PE
    stationary operand (LDWEIGHTS, fast-weight-load path) and Q^T streams
    as the 16-column moving operand, yielding score tiles [128(s), 16(q)]
    packed 16-s-tiles-per-PSUM-bank; two banks cover a pair's 32 s-tiles.
  - ScalarE applies exp(SCALE*s) over a whole [128, 512] bank (batching 2
    pairs x 16 s-tiles per instruction), writing bf16 E^T straight into the
    layout the PV matmul wants — no PE transposes, no PSUM->SBUF copies.
  - PV: per pair, 32 matmuls with the E^T s-tile slice [128, 16] stationary
    and the V tile [128, 257] moving, accumulating attn_raw (+ denominator
    in column 256) in PSUM; the active block closes the accumulation.
  - VectorE takes 1/denominator from pv[:, 256] and scales pv[:, 0:256]
    into the fp32 output tile.
The softmax max-subtraction is skipped: scaled scores are ~N(0,1) here so
exp() cannot overflow, and the result is mathematically identical.
prior_mask is all-ones per the problem spec; a numpy fallback handles the
(never expected) general case.
"""

import contextlib

import numpy as np
import ml_dtypes

import concourse.bacc as bacc
import concourse.mybir as mybir
import concourse.tile as tile
from concourse.bass_utils import run_bass_kernel_spmd

B, H, QL, SP, D = 8, 16, 16, 4096, 256
SA = SP + QL                # score columns incl. folded-in active block
SCALE = float(D) ** -0.5
N_CORES = 8
HPC = H // N_CORES          # heads per core
NP = B * HPC                # (b,h) pairs per core = 16
G = 2                       # pairs per group
NG = NP // G                # 8 groups
NST = SP // 128             # 32 V s-tiles per pair
HT = NST // 2               # s-tiles per PSUM score bank (16)
GQ = G * QL                 # score-bank q columns per s-tile (32)
DV = D + 1                  # V columns incl. the ones-column

F32 = mybir.dt.float32
BF16 = mybir.dt.bfloat16
EXP = mybir.ActivationFunctionType.Exp
BF = ml_dtypes.bfloat16

_compiled = None


def _build(loop_n=None, kt_bufs=11, v_bufs=5, esb=6, osb=4, va_bufs=2,
           ps_s_bufs=4, pv_bufs=3, v_split=False, unroll=1, ring3=False,
           sa_in_s=False, g1=False):
    nc = bacc.Bacc(
        "TRN2",
        target_bir_lowering=False,
        debug=False,
        num_devices=N_CORES,
    )
    kt_d = nc.dram_tensor("kt", [NP, 2, 128, SA], BF16, kind="ExternalInput").ap()
    v_d = nc.dram_tensor("v", [NP, 128, NST, DV], BF16, kind="ExternalInput").ap()
    qt_d = nc.dram_tensor("qt", [128, 2, NP, QL], BF16, kind="ExternalInput").ap()
    va_d = nc.dram_tensor("va", [QL, NP, DV], BF16, kind="ExternalInput").ap()
    out_d = nc.dram_tensor("out", [NP, QL, D], F32, kind="ExternalOutput").ap()

    with tile.TileContext(nc) as tc:
        with (
            tc.tile_pool(name="ktsb", bufs=kt_bufs) as ktp,
            tc.tile_pool(name="vsb", bufs=v_bufs) as vp,
            tc.tile_pool(name="smalls", bufs=2) as smallp,
            tc.tile_pool(name="vasb", bufs=va_bufs) as vasp,
            tc.tile_pool(name="esb", bufs=esb) as esbp,
            tc.tile_pool(name="osb", bufs=osb) as osbp,
            tc.tile_pool(name="stat", bufs=4) as statp,
            tc.tile_pool(name="ps_s", bufs=ps_s_bufs, space="PSUM") as ps_s,
            tc.tile_pool(name="ps_sa", bufs=1, space="PSUM") as ps_sa,
            tc.tile_pool(name="ps_pv", bufs=pv_bufs, space="PSUM") as ps_pv,
        ):
            if loop_n is not None and loop_n % unroll == 0:
                n_hw, n_body = loop_n // unroll, unroll
            elif loop_n is not None:
                n_hw, n_body = loop_n, 1
            else:
                n_hw, n_body = None, 1
            loop_cm = (tc.For_i(0, n_hw, 1) if n_hw is not None
                       else contextlib.nullcontext())
            with loop_cm:
             for _body in range(n_body):
              qt_sb = smallp.tile([128, 2, NP, QL], BF16, tag="qt")
              nc.scalar.dma_start(out=qt_sb, in_=qt_d)
              va_sb = vasp.tile([QL, NP, DV], BF16, tag="va")
              nc.scalar.dma_start(out=va_sb, in_=va_d)

              if g1:
                for p in range(NP):
                  kt0 = ktp.tile([128, SA], BF16, tag="kt")
                  nc.sync.dma_start(out=kt0, in_=kt_d[p, 0])
                  kt1 = ktp.tile([128, SA], BF16, tag="kt")
                  nc.sync.dma_start(out=kt1, in_=kt_d[p, 1])
                  v_sb = vp.tile([128, NST, DV], BF16, tag="v")
                  nc.scalar.dma_start(out=v_sb, in_=v_d[p])

                  kth = (kt0, kt1)
                  ets = []
                  for half in range(2):
                      st_ps = ps_s.tile([128, HT * QL], F32, tag="s")
                      for ti in range(HT):
                          t = half * HT + ti
                          nc.tensor.matmul(
                              st_ps[:, ti * QL:(ti + 1) * QL],
                              kt0[:, t * 128:(t + 1) * 128],
                              qt_sb[:, 0, p, :],
                              start=True, stop=False, skip_group_check=True)
                          nc.tensor.matmul(
                              st_ps[:, ti * QL:(ti + 1) * QL],
                              kt1[:, t * 128:(t + 1) * 128],
                              qt_sb[:, 1, p, :],
                              start=False, stop=True, skip_group_check=True)
                      e_t = esbp.tile([128, HT * QL], BF16, tag="e")
                      nc.scalar.activation(e_t, st_ps, EXP, scale=SCALE)
                      ets.append(e_t)

                  sa_ps = ps_s.tile([QL, QL], F32, tag="s", name="sa_g1")
                  nc.tensor.matmul(sa_ps, kt0[:, SP:SA], qt_sb[:, 0, p, :],
                                   start=True, stop=False,
                                   skip_group_check=True)
                  nc.tensor.matmul(sa_ps, kt1[:, SP:SA], qt_sb[:, 1, p, :],
                                   start=False, stop=True,
                                   skip_group_check=True)
                  ea_t = esbp.tile([QL, QL], BF16, tag="ea")
                  nc.scalar.activation(ea_t, sa_ps, EXP, scale=SCALE)

                  pv_ps = ps_pv.tile([QL, DV], F32, tag="pv")
                  for t in range(NST):
                      nc.tensor.matmul(
                          pv_ps, ets[t // HT][:, (t % HT) * QL:
                                              (t % HT + 1) * QL],
                          v_sb[:, t, :],
                          start=(t == 0), stop=False, skip_group_check=True)
                  nc.tensor.matmul(pv_ps, ea_t, va_sb[:, p, :],
                                   start=False, stop=True,
                                   skip_group_check=True)

                  rec = statp.tile([QL, 1], F32, tag="rec")
                  nc.vector.reciprocal(rec, pv_ps[:, D:DV])
                  o_sb = osbp.tile([QL, D], F32, tag="o")
                  nc.vector.tensor_scalar_mul(o_sb, pv_ps[:, 0:D], rec)
                  nc.gpsimd.dma_start(out=out_d[p], in_=o_sb)
                continue

              for grp in range(NG):
                  pairs = list(range(grp * G, (grp + 1) * G))
                  p0 = pairs[0]
                  kts, vs = [], []
                  for p in pairs:
                      kt0 = ktp.tile([128, SA], BF16, tag="kt")
                      nc.sync.dma_start(out=kt0, in_=kt_d[p, 0])
                      if v_split:
                          vh0 = vp.tile([128, HT, DV], BF16, tag="v")
                          nc.scalar.dma_start(out=vh0, in_=v_d[p, :, 0:HT, :])
                      kt1 = ktp.tile([128, SA], BF16, tag="kt")
                      (nc.gpsimd if ring3 else nc.sync).dma_start(
                          out=kt1, in_=kt_d[p, 1])
                      kts.append((kt0, kt1))
                      if v_split:
                          vh1 = vp.tile([128, HT, DV], BF16, tag="v")
                          nc.scalar.dma_start(out=vh1, in_=v_d[p, :, HT:NST, :])
                          vs.append((vh0, vh1))
                      else:
                          v_sb = vp.tile([128, NST, DV], BF16, tag="v")
                          nc.scalar.dma_start(out=v_sb, in_=v_d[p])
                          vs.append(v_sb)

                  # ---- transposed scores + exp ----------------------------
                  ets = []
                  for half in range(2):
                      st_ps = ps_s.tile([128, HT * GQ], F32, tag="s")
                      for ti in range(HT):
                          t = half * HT + ti
                          for g in range(G):
                              nc.tensor.matmul(
                                  st_ps[:, ti * GQ + g * QL:
                                        ti * GQ + (g + 1) * QL],
                                  kts[g][0][:, t * 128:(t + 1) * 128],
                                  qt_sb[:, 0, p0 + g, :],
                                  start=True, stop=False,
                                  skip_group_check=True)
                              nc.tensor.matmul(
                                  st_ps[:, ti * GQ + g * QL:
                                        ti * GQ + (g + 1) * QL],
                                  kts[g][1][:, t * 128:(t + 1) * 128],
                                  qt_sb[:, 1, p0 + g, :],
                                  start=False, stop=True,
                                  skip_group_check=True)
                      e_t = esbp.tile([128, HT * GQ], BF16, tag="e")
                      nc.scalar.activation(e_t, st_ps, EXP, scale=SCALE)
                      ets.append(e_t)

                  # ---- active scores + exp --------------------------------
                  if sa_in_s:
                      sa_ps = ps_s.tile([QL, GQ], F32, tag="s", name="sa_ps")
                  else:
                      sa_ps = ps_sa.tile([QL, GQ], F32, tag="sa")
                  for g in range(G):
                      nc.tensor.matmul(
                          sa_ps[:, g * QL:(g + 1) * QL],
                          kts[g][0][:, SP:SA], qt_sb[:, 0, p0 + g, :],
                          start=True, stop=False, skip_group_check=True)
                      nc.tensor.matmul(
                          sa_ps[:, g * QL:(g + 1) * QL],
                          kts[g][1][:, SP:SA], qt_sb[:, 1, p0 + g, :],
                          start=False, stop=True, skip_group_check=True)
                  ea_t = esbp.tile([QL, GQ], BF16, tag="ea")
                  nc.scalar.activation(ea_t, sa_ps, EXP, scale=SCALE)

                  # ---- PV (+ denominator via the ones-column) -------------
                  pv_ps = ps_pv.tile([2 * 32, DV], F32, tag="pv")
                  for g in range(G):
                      for t in range(NST):
                          nc.tensor.matmul(
                              pv_ps[g * 32:g * 32 + QL, :],
                              ets[t // HT][:, (t % HT) * GQ + g * QL:
                                           (t % HT) * GQ + (g + 1) * QL],
                              (vs[g][t // HT][:, t % HT, :] if v_split
                               else vs[g][:, t, :]),
                              start=(t == 0), stop=False,
                              skip_group_check=True)
                      nc.tensor.matmul(
                          pv_ps[g * 32:g * 32 + QL, :],
                          ea_t[:, g * QL:(g + 1) * QL],
                          va_sb[:, p0 + g, :],
                          start=False, stop=True, skip_group_check=True)

                  # ---- normalize + store ----------------------------------
                  for g, p in enumerate(pairs):
                      rec = statp.tile([QL, 1], F32, tag="rec")
                      nc.vector.reciprocal(
                          rec, pv_ps[g * 32:g * 32 + QL, D:DV])
                      o_sb = osbp.tile([QL, D], F32, tag="o")
                      nc.vector.tensor_scalar_mul(
                          o_sb, pv_ps[g * 32:g * 32 + QL, 0:D], rec)
                      nc.gpsimd.dma_start(out=out_d[p], in_=o_sb)

    nc.compile()
    return nc


def _get_compiled():
    global _compiled
    if _compiled is None:
        _compiled = _build()
    return _compiled


def make_in_maps(Q, K_prior, V_prior, K_active, V_active):
    in_maps = []
    for c in range(N_CORES):
        hs = slice(c * HPC, (c + 1) * HPC)
        kc = np.concatenate(
            [K_prior[:, hs], K_active[:, hs]], axis=2
        ).reshape(NP, SA, 2, 128)
        kt = np.ascontiguousarray(kc.transpose(0, 2, 3, 1)).astype(BF)
        v = np.ones((NP, 128, NST, DV), dtype=BF)
        v[:, :, :, :D] = V_prior[:, hs].reshape(
            NP, NST, 128, D).transpose(0, 2, 1, 3).astype(BF)
        qc = Q[:, hs].reshape(NP, QL, 2, 128)
        qt = np.ascontiguousarray(qc.transpose(3, 2, 0, 1)).astype(BF)
        va = np.ones((QL, NP, DV), dtype=BF)
        va[:, :, :D] = V_active[:, hs].reshape(
            NP, QL, D).transpose(1, 0, 2).astype(BF)
        in_maps.append({"kt": kt, "v": v, "qt": qt, "va": va})
    return in_maps


def gather_out(per_core_outs):
    full = np.stack(per_core_outs, axis=0).reshape(N_CORES, B, HPC, QL, D)
    return np.ascontiguousarray(
        full.transpose(1, 0, 2, 3, 4).reshape(B, H, QL, D))


def _numpy_fallback(Q, K_prior, V_prior, K_active, V_active, prior_mask):
    ps = np.einsum("bhqd,bhkd->bhqk", Q, K_prior) * SCALE
    as_ = np.einsum("bhqd,bhkd->bhqk", Q, K_active) * SCALE
    neg = np.finfo(np.float32).min
    ps = np.where(prior_mask, ps, neg)
    m = np.maximum(ps.max(-1, keepdims=True), as_.max(-1, keepdims=True))
    ep = np.exp(ps - m)
    ea = np.exp(as_ - m)
    den = ep.sum(-1, keepdims=True) + ea.sum(-1, keepdims=True)
    return (np.einsum("bhqk,bhkd->bhqd", (ep / den).astype(np.float32), V_prior)
            + np.einsum("bhqk,bhkd->bhqd", (ea / den).astype(np.float32),
                        V_active)).astype(np.float32)


def kernel(**inputs):
    Q = np.asarray(inputs["Q"], dtype=np.float32)
    K_prior = np.asarray(inputs["K_prior"], dtype=np.float32)
    V_prior = np.asarray(inputs["V_prior"], dtype=np.float32)
    K_active = np.asarray(inputs["K_active"], dtype=np.float32)
    V_active = np.asarray(inputs["V_active"], dtype=np.float32)
    prior_mask = np.asarray(inputs["prior_mask"])

    if not prior_mask.all():
        # Spec guarantees an all-ones mask; general masks take the slow path.
        return _numpy_fallback(Q, K_prior, V_prior, K_active, V_active,
                               prior_mask)

    nc = _get_compiled()
    res = run_bass_kernel_spmd(
        nc,
        make_in_maps(Q, K_prior, V_prior, K_active, V_active),
        core_ids=list(range(N_CORES)),
    )
    return gather_out([res.results[c]["out"] for c in range(N_CORES)])


# revision 13
# speedup vs baseline: 1.0610x; 1.0025x over previous
"""Trainium2 Bass kernel for nn_NewAttentionBlock (sparse_attention).

Joint softmax attention over a large all-ones-masked "prior" KV block
(S=4096) plus a small "active" KV block (S=16), for B=8, H=16, Q=16, D=256,
fp32 in/out.

Sharding: heads are split across the 8 NeuronCores (2 heads/core, tensor
parallel, no cross-core communication).  Each core processes its 16 (b,h)
pairs fully independently.

The problem is HBM-bandwidth bound (K_prior/V_prior = 128 MiB/core in fp32),
so K/V/Q are quantized to bf16 on the host (measured end-to-end max rel err
~6e-3 vs the fp32 reference, within the 2e-2 gate), halving DMA traffic, and
all layout work is done host-side so the device performs only the compulsory
streaming reads:

  - K^T is built on the host, d-major: kt[p, half, d, s] with the 16
    K_active rows folded in as score columns s = 4096..4111.
  - V is tiled [128(s), 32(tile), 257(d)] per pair — column 256 is a
    constant 1.0, which makes the PV matmul accumulate the softmax
    denominator sum(E) alongside E@V for free.
  - Q^T is host-transposed: [128(d), half, pair, 16(q)].

Device dataflow per 2-pair group (all matmuls bf16, fp32 PSUM accumulate):
  - scores are computed TRANSPOSED: the K^T 128x128 slice is the PE
    stationary operand (LDWEIGHTS, fast-weight-load path) and Q^T streams
    as the 16-column moving operand, yielding score tiles [128(s), 16(q)]
    packed 16-s-tiles-per-PSUM-bank; two banks cover a pair's 32 s-tiles.
  - ScalarE applies exp(SCALE*s) over a whole [128, 512] bank (batching 2
    pairs x 16 s-tiles per instruction), writing bf16 E^T straight into the
    layout the PV matmul wants — no PE transposes, no PSUM->SBUF copies.
  - PV: per pair, 32 matmuls with the E^T s-tile slice [128, 16] stationary
    and the V tile [128, 257] moving, accumulating attn_raw (+ denominator
    in column 256) in PSUM; the active block closes the accumulation.
  - VectorE takes 1/denominator from pv[:, 256] and scales pv[:, 0:256]
    into the fp32 output tile.
The softmax max-subtraction is skipped: scaled scores are ~N(0,1) here so
exp() cannot overflow, and the result is mathematically identical.
prior_mask is all-ones per the problem spec; a numpy fallback handles the
(never expected) general case.
"""

import contextlib

import numpy as np
import ml_dtypes

import concourse.bacc as bacc
import concourse.mybir as mybir
import concourse.tile as tile
from concourse.bass_utils import run_bass_kernel_spmd

B, H, QL, SP, D = 8, 16, 16, 4096, 256
SA = SP + QL                # score columns incl. folded-in active block
SCALE = float(D) ** -0.5
N_CORES = 8
HPC = H // N_CORES          # heads per core
NP = B * HPC                # (b,h) pairs per core = 16
G = 2                       # pairs per group
NG = NP // G                # 8 groups
NST = SP // 128             # 32 V s-tiles per pair
HT = NST // 2               # s-tiles per PSUM score bank (16)
GQ = G * QL                 # score-bank q columns per s-tile (32)
DV = D + 1                  # V columns incl. the ones-column

F32 = mybir.dt.float32
BF16 = mybir.dt.bfloat16
EXP = mybir.ActivationFunctionType.Exp
BF = ml_dtypes.bfloat16

_compiled = None


def _build(loop_n=None, kt_bufs=11, v_bufs=5, esb=6, osb=4, va_bufs=2,
           ps_s_bufs=4, pv_bufs=3, v_split=False, unroll=1, ring3=False,
           sa_in_s=False, g1=False, act_q=1, alt_rings=False):
    nc = bacc.Bacc(
        "TRN2",
        target_bir_lowering=False,
        debug=False,
        num_devices=N_CORES,
    )
    kt_d = nc.dram_tensor("kt", [NP, 2, 128, SA], BF16, kind="ExternalInput").ap()
    v_d = nc.dram_tensor("v", [NP, 128, NST, DV], BF16, kind="ExternalInput").ap()
    qt_d = nc.dram_tensor("qt", [128, 2, NP, QL], BF16, kind="ExternalInput").ap()
    va_d = nc.dram_tensor("va", [QL, NP, DV], BF16, kind="ExternalInput").ap()
    out_d = nc.dram_tensor("out", [NP, QL, D], F32, kind="ExternalOutput").ap()

    with tile.TileContext(nc) as tc:
        with (
            tc.tile_pool(name="ktsb", bufs=kt_bufs) as ktp,
            tc.tile_pool(name="vsb", bufs=v_bufs) as vp,
            tc.tile_pool(name="smalls", bufs=2) as smallp,
            tc.tile_pool(name="vasb", bufs=va_bufs) as vasp,
            tc.tile_pool(name="esb", bufs=esb) as esbp,
            tc.tile_pool(name="osb", bufs=osb) as osbp,
            tc.tile_pool(name="stat", bufs=4) as statp,
            tc.tile_pool(name="ps_s", bufs=ps_s_bufs, space="PSUM") as ps_s,
            tc.tile_pool(name="ps_sa", bufs=1, space="PSUM") as ps_sa,
            tc.tile_pool(name="ps_pv", bufs=pv_bufs, space="PSUM") as ps_pv,
        ):
            if loop_n is not None and loop_n % unroll == 0:
                n_hw, n_body = loop_n // unroll, unroll
            elif loop_n is not None:
                n_hw, n_body = loop_n, 1
            else:
                n_hw, n_body = None, 1
            loop_cm = (tc.For_i(0, n_hw, 1) if n_hw is not None
                       else contextlib.nullcontext())
            with loop_cm:
             for _body in range(n_body):
              qt_sb = smallp.tile([128, 2, NP, QL], BF16, tag="qt")
              nc.scalar.dma_start(out=qt_sb, in_=qt_d)
              va_sb = vasp.tile([QL, NP, DV], BF16, tag="va")
              nc.scalar.dma_start(out=va_sb, in_=va_d)

              if g1:
                for p in range(NP):
                  kt0 = ktp.tile([128, SA], BF16, tag="kt")
                  nc.sync.dma_start(out=kt0, in_=kt_d[p, 0])
                  kt1 = ktp.tile([128, SA], BF16, tag="kt")
                  nc.sync.dma_start(out=kt1, in_=kt_d[p, 1])
                  v_sb = vp.tile([128, NST, DV], BF16, tag="v")
                  nc.scalar.dma_start(out=v_sb, in_=v_d[p])

                  kth = (kt0, kt1)
                  ets = []
                  for half in range(2):
                      st_ps = ps_s.tile([128, HT * QL], F32, tag="s")
                      for ti in range(HT):
                          t = half * HT + ti
                          nc.tensor.matmul(
                              st_ps[:, ti * QL:(ti + 1) * QL],
                              kt0[:, t * 128:(t + 1) * 128],
                              qt_sb[:, 0, p, :],
                              start=True, stop=False, skip_group_check=True)
                          nc.tensor.matmul(
                              st_ps[:, ti * QL:(ti + 1) * QL],
                              kt1[:, t * 128:(t + 1) * 128],
                              qt_sb[:, 1, p, :],
                              start=False, stop=True, skip_group_check=True)
                      e_t = esbp.tile([128, HT * QL], BF16, tag="e")
                      nc.scalar.activation(e_t, st_ps, EXP, scale=SCALE)
                      ets.append(e_t)

                  sa_ps = ps_s.tile([QL, QL], F32, tag="s", name="sa_g1")
                  nc.tensor.matmul(sa_ps, kt0[:, SP:SA], qt_sb[:, 0, p, :],
                                   start=True, stop=False,
                                   skip_group_check=True)
                  nc.tensor.matmul(sa_ps, kt1[:, SP:SA], qt_sb[:, 1, p, :],
                                   start=False, stop=True,
                                   skip_group_check=True)
                  ea_t = esbp.tile([QL, QL], BF16, tag="ea")
                  nc.scalar.activation(ea_t, sa_ps, EXP, scale=SCALE)

                  pv_ps = ps_pv.tile([QL, DV], F32, tag="pv")
                  for t in range(NST):
                      nc.tensor.matmul(
                          pv_ps, ets[t // HT][:, (t % HT) * QL:
                                              (t % HT + 1) * QL],
                          v_sb[:, t, :],
                          start=(t == 0), stop=False, skip_group_check=True)
                  nc.tensor.matmul(pv_ps, ea_t, va_sb[:, p, :],
                                   start=False, stop=True,
                                   skip_group_check=True)

                  rec = statp.tile([QL, 1], F32, tag="rec")
                  nc.vector.reciprocal(rec, pv_ps[:, D:DV])
                  o_sb = osbp.tile([QL, D], F32, tag="o")
                  nc.vector.tensor_scalar_mul(o_sb, pv_ps[:, 0:D], rec)
                  nc.gpsimd.dma_start(out=out_d[p], in_=o_sb)
                continue

              for grp in range(NG):
                  pairs = list(range(grp * G, (grp + 1) * G))
                  p0 = pairs[0]
                  kts, vs = [], []
                  for p in pairs:
                      if alt_rings and p % 2 == 1:
                          keng, veng = nc.scalar, nc.sync
                      else:
                          keng, veng = nc.sync, nc.scalar
                      kt0 = ktp.tile([128, SA], BF16, tag="kt")
                      keng.dma_start(out=kt0, in_=kt_d[p, 0])
                      if v_split:
                          vh0 = vp.tile([128, HT, DV], BF16, tag="v")
                          nc.scalar.dma_start(out=vh0, in_=v_d[p, :, 0:HT, :])
                      kt1 = ktp.tile([128, SA], BF16, tag="kt")
                      (nc.gpsimd if ring3 else keng).dma_start(
                          out=kt1, in_=kt_d[p, 1])
                      kts.append((kt0, kt1))
                      if v_split:
                          vh1 = vp.tile([128, HT, DV], BF16, tag="v")
                          nc.scalar.dma_start(out=vh1, in_=v_d[p, :, HT:NST, :])
                          vs.append((vh0, vh1))
                      else:
                          v_sb = vp.tile([128, NST, DV], BF16, tag="v")
                          veng.dma_start(out=v_sb, in_=v_d[p])
                          vs.append(v_sb)

                  # ---- transposed scores + exp ----------------------------
                  ets = []
                  for half in range(2):
                      st_ps = ps_s.tile([128, HT * GQ], F32, tag="s")
                      for ti in range(HT):
                          t = half * HT + ti
                          for g in range(G):
                              nc.tensor.matmul(
                                  st_ps[:, ti * GQ + g * QL:
                                        ti * GQ + (g + 1) * QL],
                                  kts[g][0][:, t * 128:(t + 1) * 128],
                                  qt_sb[:, 0, p0 + g, :],
                                  start=True, stop=False,
                                  skip_group_check=True)
                              nc.tensor.matmul(
                                  st_ps[:, ti * GQ + g * QL:
                                        ti * GQ + (g + 1) * QL],
                                  kts[g][1][:, t * 128:(t + 1) * 128],
                                  qt_sb[:, 1, p0 + g, :],
                                  start=False, stop=True,
                                  skip_group_check=True)
                      e_t = esbp.tile([128, HT * GQ], BF16, tag="e")
                      qw = HT * GQ // act_q
                      for qq in range(act_q):
                          nc.scalar.activation(
                              e_t[:, qq * qw:(qq + 1) * qw],
                              st_ps[:, qq * qw:(qq + 1) * qw],
                              EXP, scale=SCALE)
                      ets.append(e_t)

                  # ---- active scores + exp --------------------------------
                  if sa_in_s:
                      sa_ps = ps_s.tile([QL, GQ], F32, tag="s", name="sa_ps")
                  else:
                      sa_ps = ps_sa.tile([QL, GQ], F32, tag="sa")
                  for g in range(G):
                      nc.tensor.matmul(
                          sa_ps[:, g * QL:(g + 1) * QL],
                          kts[g][0][:, SP:SA], qt_sb[:, 0, p0 + g, :],
                          start=True, stop=False, skip_group_check=True)
                      nc.tensor.matmul(
                          sa_ps[:, g * QL:(g + 1) * QL],
                          kts[g][1][:, SP:SA], qt_sb[:, 1, p0 + g, :],
                          start=False, stop=True, skip_group_check=True)
                  ea_t = esbp.tile([QL, GQ], BF16, tag="ea")
                  nc.scalar.activation(ea_t, sa_ps, EXP, scale=SCALE)

                  # ---- PV (+ denominator via the ones-column) -------------
                  pv_ps = ps_pv.tile([2 * 32, DV], F32, tag="pv")
                  for g in range(G):
                      for t in range(NST):
                          nc.tensor.matmul(
                              pv_ps[g * 32:g * 32 + QL, :],
                              ets[t // HT][:, (t % HT) * GQ + g * QL:
                                           (t % HT) * GQ + (g + 1) * QL],
                              (vs[g][t // HT][:, t % HT, :] if v_split
                               else vs[g][:, t, :]),
                              start=(t == 0), stop=False,
                              skip_group_check=True)
                      nc.tensor.matmul(
                          pv_ps[g * 32:g * 32 + QL, :],
                          ea_t[:, g * QL:(g + 1) * QL],
                          va_sb[:, p0 + g, :],
                          start=False, stop=True, skip_group_check=True)

                  # ---- normalize + store ----------------------------------
                  for g, p in enumerate(pairs):
                      rec = statp.tile([QL, 1], F32, tag="rec")
                      nc.vector.reciprocal(
                          rec, pv_ps[g * 32:g * 32 + QL, D:DV])
                      o_sb = osbp.tile([QL, D], F32, tag="o")
                      nc.vector.tensor_scalar_mul(
                          o_sb, pv_ps[g * 32:g * 32 + QL, 0:D], rec)
                      nc.gpsimd.dma_start(out=out_d[p], in_=o_sb)

    nc.compile()
    return nc


def _get_compiled():
    global _compiled
    if _compiled is None:
        _compiled = _build()
    return _compiled


def make_in_maps(Q, K_prior, V_prior, K_active, V_active):
    in_maps = []
    for c in range(N_CORES):
        hs = slice(c * HPC, (c + 1) * HPC)
        kc = np.concatenate(
            [K_prior[:, hs], K_active[:, hs]], axis=2
        ).reshape(NP, SA, 2, 128)
        kt = np.ascontiguousarray(kc.transpose(0, 2, 3, 1)).astype(BF)
        v = np.ones((NP, 128, NST, DV), dtype=BF)
        v[:, :, :, :D] = V_prior[:, hs].reshape(
            NP, NST, 128, D).transpose(0, 2, 1, 3).astype(BF)
        qc = Q[:, hs].reshape(NP, QL, 2, 128)
        qt = np.ascontiguousarray(qc.transpose(3, 2, 0, 1)).astype(BF)
        va = np.ones((QL, NP, DV), dtype=BF)
        va[:, :, :D] = V_active[:, hs].reshape(
            NP, QL, D).transpose(1, 0, 2).astype(BF)
        in_maps.append({"kt": kt, "v": v, "qt": qt, "va": va})
    return in_maps


def gather_out(per_core_outs):
    full = np.stack(per_core_outs, axis=0).reshape(N_CORES, B, HPC, QL, D)
    return np.ascontiguousarray(
        full.transpose(1, 0, 2, 3, 4).reshape(B, H, QL, D))


def _numpy_fallback(Q, K_prior, V_prior, K_active, V_active, prior_mask):
    ps = np.einsum("bhqd,bhkd->bhqk", Q, K_prior) * SCALE
    as_ = np.einsum("bhqd,bhkd->bhqk", Q, K_active) * SCALE
    neg = np.finfo(np.float32).min
    ps = np.where(prior_mask, ps, neg)
    m = np.maximum(ps.max(-1, keepdims=True), as_.max(-1, keepdims=True))
    ep = np.exp(ps - m)
    ea = np.exp(as_ - m)
    den = ep.sum(-1, keepdims=True) + ea.sum(-1, keepdims=True)
    return (np.einsum("bhqk,bhkd->bhqd", (ep / den).astype(np.float32), V_prior)
            + np.einsum("bhqk,bhkd->bhqd", (ea / den).astype(np.float32),
                        V_active)).astype(np.float32)


def kernel(**inputs):
    Q = np.asarray(inputs["Q"], dtype=np.float32)
    K_prior = np.asarray(inputs["K_prior"], dtype=np.float32)
    V_prior = np.asarray(inputs["V_prior"], dtype=np.float32)
    K_active = np.asarray(inputs["K_active"], dtype=np.float32)
    V_active = np.asarray(inputs["V_active"], dtype=np.float32)
    prior_mask = np.asarray(inputs["prior_mask"])

    if not prior_mask.all():
        # Spec guarantees an all-ones mask; general masks take the slow path.
        return _numpy_fallback(Q, K_prior, V_prior, K_active, V_active,
                               prior_mask)

    nc = _get_compiled()
    res = run_bass_kernel_spmd(
        nc,
        make_in_maps(Q, K_prior, V_prior, K_active, V_active),
        core_ids=list(range(N_CORES)),
    )
    return gather_out([res.results[c]["out"] for c in range(N_CORES)])
